# revision 12
# baseline (speedup 1.0000x reference)
"""CliffordLayerNorm Trainium2 kernel.

x: [16, 4096, 1024] fp32. Each row's 1024 features = 4 blocks of 256
multivector components; components are grouped into 9 grades by popcount of
their index within the block.  Per (token, block, grade): mean/var, then
out = (x - mean) * w[g] * rsqrt(var + eps) + b[g].

Strategy (per NeuronCore, data-parallel over tokens across 8 cores):
  Software-pipelined over groups of 512 tokens (16 groups/core), with the
  back end lagged 2 groups behind the front end so the tiny stats chain is
  never on the PE critical path:

  front end (group g):
    1. DMA in [128 tok, 4 j, 1024 feat] (prefetched one group ahead).
    2. PE-transposes (f32r, 1.5 cyc/row) each 128x128 chunk into PSUM.
    3. ACT copies PSUM -> SBUF xT (bf16); squares on ACT+Pool -> sqT.
    4. PE stats matmuls vs the 1/count grade matrix give per-(block,grade)
       mean and mean-of-squares: PSUM S12 [36, 2, 512].
    5. Chain on ACT/DVE/Pool: rstd = rsqrt(|var+eps|), c2n = b/w - mean*rstd.
  back end (group g-2), PE scatter matmuls interleaved into the front end's
  transpose/stats stream so the PE never waits on PSUM drain:
    6. PE scatter-matmuls expand rstd/c2n to per-element scale A / shift B.
    7. DVE 2-pass AXPY per j: tmp = x*A; out = tmp + B (in-place), DMA out.
"""

import os
import sys

if "/opt/trn_rl_repo" not in sys.path:
    sys.path.insert(0, "/opt/trn_rl_repo")

import numpy as np

BLOCK_BITS = 8
MV = 256
NG = 9
NB = 4
D = 1024
EPS = 1e-5
N_CORES = 8
TOTAL_TOKENS = 16 * 4096
TOK_PER_CORE = TOTAL_TOKENS // N_CORES  # 8192

GROUP_T = 512          # tokens per stats group
LAG = 2                # back end lags this many groups behind the front end


def _grade(m):
    return bin(m).count("1")


def _build_consts():
    import math
    counts = np.array([math.comb(8, g) for g in range(NG)], dtype=np.float32)

    # G_mean[h][i, b*9+g] = 1/count_g  for chunk h (features 128h..128h+127)
    gmean = np.zeros((8, 128, 36), dtype=np.float32)
    for h in range(8):
        b = h // 2
        for i in range(128):
            m = (h % 2) * 128 + i
            g = _grade(m)
            gmean[h, i, b * 9 + g] = 1.0 / counts[g]

    # G01[b*9+g, c] = 1 if feature c belongs to (block b, grade g)
    g01 = np.zeros((36, D), dtype=np.float32)
    for c in range(D):
        b = c // MV
        g = _grade(c % MV)
        g01[b * 9 + g, c] = 1.0

    # rstd mask: count-1 grades (0 and 8) have centered value exactly 0 in
    # the reference, so any scale works -- force rstd=0 there to avoid
    # amplifying f32r rounding by rsqrt(eps).
    mask = np.ones((36, 1), dtype=np.float32)
    for b in range(NB):
        mask[b * 9 + 0, 0] = 0.0
        mask[b * 9 + 8, 0] = 0.0
    return gmean, g01, mask


def build_nc(tok_per_core=TOK_PER_CORE):
    import concourse.bass as bass
    import concourse.tile as tile
    from concourse import bacc, mybir

    f32 = mybir.dt.float32
    f32r = mybir.dt.float32r
    bf16 = mybir.dt.bfloat16
    AF = mybir.ActivationFunctionType
    ALU = mybir.AluOpType

    gmean_np, g01_np, mask_np = _build_consts()
    n_groups = tok_per_core // GROUP_T
    assert tok_per_core % GROUP_T == 0

    nc = bacc.Bacc()
    x_d = nc.dram_tensor("x", [tok_per_core, D], f32, kind="ExternalInput")
    w_d = nc.dram_tensor("weight", [NG], f32, kind="ExternalInput")
    b_d = nc.dram_tensor("bias", [NG], f32, kind="ExternalInput")
    out_d = nc.dram_tensor("out", [tok_per_core, D], f32, kind="ExternalOutput")

    gmean_dram = nc.inline_tensor(gmean_np, name="gmean_const")
    g01_dram = nc.inline_tensor(g01_np, name="g01_const")
    ident_dram = nc.inline_tensor(np.eye(128, dtype=np.float32), name="ident_const")
    mask_dram = nc.inline_tensor(mask_np, name="mask_const")

    from contextlib import ExitStack

    with tile.TileContext(nc) as tc, ExitStack() as ctx:
        consts = ctx.enter_context(tc.tile_pool(name="consts", bufs=1))
        xg_pool = ctx.enter_context(tc.tile_pool(name="xg", bufs=5))
        xb_pool = ctx.enter_context(tc.tile_pool(name="xb", bufs=2))
        xt_pool = ctx.enter_context(tc.tile_pool(name="xt", bufs=2))
        sqt_pool = ctx.enter_context(tc.tile_pool(name="sqt", bufs=2))
        tmp_pool = ctx.enter_context(tc.tile_pool(name="tmp", bufs=2))
        small_pool = ctx.enter_context(tc.tile_pool(name="small", bufs=2))
        ps_xt = ctx.enter_context(tc.tile_pool(name="ps_xt", bufs=2, space="PSUM"))
        ps_stats = ctx.enter_context(tc.tile_pool(name="ps_st", bufs=1, space="PSUM"))
        ps_a = ctx.enter_context(tc.tile_pool(name="ps_a", bufs=1, space="PSUM"))
        ps_b = ctx.enter_context(tc.tile_pool(name="ps_b", bufs=1, space="PSUM"))

        # ---- constants into SBUF ----
        # All const DMAs go through gpsimd (SWDGE, single queue -> single
        # semaphore) so downstream compute needs at most one new wait.
        # bf16 identity + bf16 x: bf16 transposes run 1 cyc/row (vs 2 for
        # fp32) and bf16 stationaries load via a separate, overlapped
        # LDWEIGHTS (4-byte stationaries are self-loading and serialize)
        ident_f = consts.tile([128, 128], f32)
        nc.gpsimd.dma_start(out=ident_f, in_=ident_dram[:])
        ident = consts.tile([128, 128], bf16)
        nc.vector.tensor_scalar_mul(ident, ident_f, 1.0)

        gmean_f = consts.tile([128, 8, 36], f32)
        nc.gpsimd.dma_start(out=gmean_f, in_=gmean_dram[:].rearrange("h p c -> p h c"))

        g01_sb = consts.tile([36, D], f32)
        nc.gpsimd.dma_start(out=g01_sb, in_=g01_dram[:])

        # weight/bias broadcast to 36 partitions: partition p = b*9+g reads w[g]
        w36 = consts.tile([36, 1], f32)
        b36 = consts.tile([36, 1], f32)
        wap = w_d[:]
        bap = b_d[:]
        nc.gpsimd.dma_start(
            out=w36, in_=bass.AP(tensor=wap.tensor, offset=wap.offset,
                                 ap=[[0, NB]] + list(wap.ap)))
        nc.gpsimd.dma_start(
            out=b36, in_=bass.AP(tensor=bap.tensor, offset=bap.offset,
                                 ap=[[0, NB]] + list(bap.ap)))

        mask36 = consts.tile([36, 1], f32)
        nc.gpsimd.dma_start(out=mask36, in_=mask_dram[:])
        # eps + 1e38*(1-mask): count-1 grades get a huge bias so the fused
        # abs-rsqrt returns ~1e-19 (i.e. rstd ~= 0) for them
        eps36 = consts.tile([36, 1], f32)
        nc.vector.tensor_scalar(
            out=eps36, in0=mask36, scalar1=-1e38, scalar2=1e38 + EPS,
            op0=ALU.mult, op1=ALU.add)
        gmean_sb = consts.tile([128, 8, 36], bf16)
        nc.vector.tensor_scalar_mul(gmean_sb, gmean_f, 1.0)
        rw36 = consts.tile([36, 1], f32)
        nc.vector.reciprocal(rw36, w36)
        # GA[bg, c] = w[g(c)] * indicator; ga_mask additionally zeroes
        # count-1 grades (their centered value is exactly 0 in the reference)
        ga_sb = consts.tile([36, D], bf16)
        nc.vector.tensor_scalar_mul(ga_sb, g01_sb, w36)
        w36m = consts.tile([36, 1], f32)
        nc.vector.tensor_scalar_mul(w36m, w36, mask36)
        ga_mask = consts.tile([36, D], bf16)
        nc.vector.tensor_scalar_mul(ga_mask, g01_sb, w36m)
        bw36 = consts.tile([36, 1], f32)   # b/w
        nc.vector.tensor_scalar_mul(bw36, b36, rw36)
        # bw broadcast along tokens so the chain's c2n is a cheap
        # tensor_tensor on Pool (tensor_scalar with a vector scalar is a
        # microcoded slow path there)
        bwb = consts.tile([36, GROUP_T], f32)
        nc.vector.tensor_scalar(
            out=bwb, in0=g01_sb[:, 0:GROUP_T], scalar1=0.0, scalar2=bw36,
            op0=ALU.mult, op1=ALU.add)

        # ---- pipelined main loop ----
        # iteration i: prefetch dma(i+1), front end(i), back end(i-LAG)
        state = {}  # per-group tiles carried across iterations

        def dma_in(g):
            xg = xg_pool.tile([128, NB, D], f32, name="xg_t")
            nc.sync.dma_start(
                out=xg,
                in_=x_d[g * GROUP_T:(g + 1) * GROUP_T, :]
                .rearrange("(j p) d -> p j d", p=128),
            )
            state[g] = {"xg": xg}

        def front_convert(g, j, engine):
            """xb = bf16(x) for token block j (feeds the bf16 transposes)."""
            st = state[g]
            if "xb" not in st:
                st["xb"] = xb_pool.tile([128, NB, D], bf16, name="xb_t")
            if engine == "act":
                nc.scalar.copy(out=st["xb"][:, j, :], in_=st["xg"][:, j, :])
            else:
                nc.gpsimd.tensor_copy(st["xb"][:, j, :], st["xg"][:, j, :])

        def front_transposes(g, half, jj):
            """Transpose feature chunks half*4..half*4+3 for token blocks jj
            (a pair of j's -> 8 transposes into one bf16 PSUM bank)."""
            st = state[g]
            xb = st["xb"]
            if "xT" not in st:
                st["xT"] = xt_pool.tile([128, 8, GROUP_T], bf16, name="xT_t")
                st["sqT"] = sqt_pool.tile([128, 8, GROUP_T], bf16, name="sqT_t")
            j0 = jj[0]
            xt_ps = ps_xt.tile([128, 1024], bf16, name="xt_ps_t")
            for jo, j in enumerate(jj):
                for cc in range(4):
                    chunk = half * 4 + cc
                    nc.tensor.transpose(
                        xt_ps[:, jo * 512 + cc * 128:jo * 512 + (cc + 1) * 128],
                        xb[:, j, chunk * 128:(chunk + 1) * 128],
                        ident,
                    )
            nc.scalar.copy(
                out=st["xT"][:, half * 4:(half + 1) * 4,
                             j0 * 128:(j0 + 2) * 128]
                .rearrange("p c (j t) -> p c j t", j=2),
                in_=xt_ps[:].rearrange("p (j c t) -> p c j t", j=2, c=4))

        def front_square(g, j, engine):
            st = state[g]
            sl = (slice(None), slice(0, 8), slice(j * 128, (j + 1) * 128))
            if engine == "act":
                nc.scalar.square(out=st["sqT"][sl], in_=st["xT"][sl])
            else:
                nc.gpsimd.tensor_tensor(
                    out=st["sqT"][sl], in0=st["xT"][sl], in1=st["xT"][sl],
                    op=ALU.mult)

        def front_stats_x(g, hs):
            st = state[g]
            if "S12" not in st:
                st["S12"] = ps_stats.tile([36, 2, GROUP_T], f32, name="S12_t")
            for h in hs:
                nc.tensor.matmul(
                    st["S12"][:, 0, :], gmean_sb[:, h, :], st["xT"][:, h, :],
                    start=(h == 0), stop=(h == 7),
                )

        def front_stats_sq(g, jp):
            st = state[g]
            sl = slice(jp * 256, (jp + 1) * 256)
            for h in range(8):
                nc.tensor.matmul(
                    st["S12"][:, 1, sl], gmean_sb[:, h, :],
                    st["sqT"][:, h, sl],
                    start=(h == 0), stop=(h == 7),
                )

        def front_chain(g):
            st = state[g]
            S12 = st["S12"]
            mean_sb = small_pool.tile([36, GROUP_T], f32)
            nc.scalar.copy(out=mean_sb, in_=S12[:, 0, :])
            msq = small_pool.tile([36, GROUP_T], f32)
            nc.scalar.square(out=msq, in_=S12[:, 0, :])
            var_t = small_pool.tile([36, GROUP_T], f32)
            nc.vector.tensor_tensor(out=var_t, in0=S12[:, 1, :], in1=msq,
                                    op=ALU.subtract)
            # rstd = 1/sqrt(|var + eps|): abs also absorbs tiny negative var
            # from rounding (count-1 grades are masked out anyway).
            # bf16 so the scatter matmuls' stationary loads go through a
            # separate LDWEIGHTS that overlaps the previous matmul (4-byte
            # stationaries are self-loading and serialize on the PE).
            rstd_t = small_pool.tile([36, GROUP_T], bf16)
            nc.scalar.activation(rstd_t, var_t, AF.Abs_reciprocal_sqrt,
                                 bias=eps36, scale=1.0)
            c_t = small_pool.tile([36, GROUP_T], f32)
            nc.vector.tensor_tensor(out=c_t, in0=mean_sb, in1=rstd_t,
                                    op=ALU.mult)
            # c2n = b/w - mean*rstd
            c2n_t = small_pool.tile([36, GROUP_T], bf16)
            nc.gpsimd.tensor_tensor(out=c2n_t, in0=bwb, in1=c_t,
                                    op=ALU.subtract)
            st["rstd"] = rstd_t
            st["c2n"] = c2n_t

        def back_scatter_a(g, j):
            st = state[g]
            pa = ps_a.tile([128, 2, 512], f32, name="pa_t")
            for half in range(2):
                nc.tensor.matmul(
                    pa[:, half, :], st["rstd"][:, j * 128:(j + 1) * 128],
                    ga_mask[:, half * 512:(half + 1) * 512])
            st["pa"] = pa

        def back_scatter_b(g, j):
            st = state[g]
            pb = ps_b.tile([128, 2, 512], f32, name="pb_t")
            for half in range(2):
                nc.tensor.matmul(
                    pb[:, half, :], st["c2n"][:, j * 128:(j + 1) * 128],
                    ga_sb[:, half * 512:(half + 1) * 512])
            st["pb"] = pb

        def back_pass1(g, j):
            st = state[g]
            tmp = tmp_pool.tile([128, D], f32, name="tmp_t")
            nc.vector.tensor_tensor(
                out=tmp, in0=st["xg"][:, j, :],
                in1=st["pa"][:].rearrange("p a b -> p (a b)"), op=ALU.mult)
            st["tmp"] = tmp

        def back_pass2(g, j):
            st = state[g]
            nc.vector.tensor_tensor(
                out=st["xg"][:, j, :], in0=st["tmp"],
                in1=st["pb"][:].rearrange("p a b -> p (a b)"), op=ALU.add)

        def dma_out(g):
            st = state[g]
            nc.sync.dma_start(
                out=out_d[g * GROUP_T:(g + 1) * GROUP_T, :]
                .rearrange("(j p) d -> p j d", p=128),
                in_=st["xg"],
            )
            del state[g]

        dma_in(0)
        for j in range(NB):
            front_convert(0, j, "act" if j % 2 == 0 else "pool")
        for i in range(n_groups + LAG):
            F = i < n_groups        # front-end group
            b = i - LAG             # back-end group
            B = b >= 0
            if i + 1 < n_groups:
                dma_in(i + 1)
            # back-end j-blocks [A(j), B(j)] + [pass1(j), pass2(j)] are
            # emitted tight so the DVE runs its two passes back-to-back;
            # B(j)'s psum-pool wait (pass2(j-1)) is hidden by the PE filler
            # (transposes/stats of the front-end group) between blocks.
            if B:
                back_scatter_a(b, 0)
                back_scatter_b(b, 0)
                back_pass1(b, 0)
                back_pass2(b, 0)
            if F:
                front_transposes(i, 0, (0, 1))
            if B:
                back_scatter_a(b, 1)
                back_scatter_b(b, 1)
                back_pass1(b, 1)
                back_pass2(b, 1)
            if F:
                front_transposes(i, 0, (2, 3))
            if B:
                back_scatter_a(b, 2)
                back_scatter_b(b, 2)
                back_pass1(b, 2)
                back_pass2(b, 2)
            if F:
                front_transposes(i, 1, (0, 1))
                front_stats_x(i, range(0, 4))
                front_square(i, 0, "act")
                front_square(i, 1, "pool")
            if B:
                back_scatter_a(b, 3)
                back_scatter_b(b, 3)
                back_pass1(b, 3)
                back_pass2(b, 3)
            if F:
                front_transposes(i, 1, (2, 3))
                front_stats_x(i, range(4, 8))
                front_square(i, 2, "pool")
                front_square(i, 3, "pool")
                front_stats_sq(i, 0)
                front_stats_sq(i, 1)
                if i + 1 < n_groups:
                    for j in range(NB):
                        front_convert(i + 1, j, "act" if j % 2 == 0 else "pool")
            if B:
                dma_out(b)
            if F:
                front_chain(i)

    nc.finalize()
    return nc


_NC_CACHE = {}


def _get_nc(tok_per_core=TOK_PER_CORE):
    key = tok_per_core
    if key not in _NC_CACHE:
        _NC_CACHE[key] = build_nc(tok_per_core)
    return _NC_CACHE[key]


def kernel(x, weight, bias, _trace=False):
    x = np.ascontiguousarray(np.asarray(x, dtype=np.float32))
    weight = np.ascontiguousarray(np.asarray(weight, dtype=np.float32))
    bias = np.ascontiguousarray(np.asarray(bias, dtype=np.float32))
    orig_shape = x.shape
    xf = x.reshape(TOTAL_TOKENS, D)

    nc = _get_nc()
    from concourse.bass_utils import run_bass_kernel_spmd

    in_maps = [
        {
            "x": np.ascontiguousarray(xf[i * TOK_PER_CORE:(i + 1) * TOK_PER_CORE]),
            "weight": weight,
            "bias": bias,
        }
        for i in range(N_CORES)
    ]
    res = run_bass_kernel_spmd(nc, in_maps, core_ids=list(range(N_CORES)),
                               trace=_trace)
    out = np.concatenate([r["out"] for r in res.results], axis=0)
    if _trace:
        kernel.last_result = res
    return out.reshape(orig_shape)


# revision 13
# speedup vs baseline: 1.0968x; 1.0968x over previous
"""CliffordLayerNorm Trainium2 kernel.

x: [16, 4096, 1024] fp32. Each row's 1024 features = 4 blocks of 256
multivector components; components are grouped into 9 grades by popcount of
their index within the block.  Per (token, block, grade): mean/var, then
out = (x - mean) * w[g] * rsqrt(var + eps) + b[g].

Strategy (per NeuronCore, data-parallel over tokens across 8 cores):
  Software-pipelined over groups of 512 tokens (16 groups/core), with the
  back end lagged 2 groups behind the front end so the tiny stats chain is
  never on the PE critical path:

  front end (group g):
    1. DMA in [128 tok, 4 j, 1024 feat] (prefetched one group ahead).
    2. PE-transposes (f32r, 1.5 cyc/row) each 128x128 chunk into PSUM.
    3. ACT copies PSUM -> SBUF xT (bf16); squares on ACT+Pool -> sqT.
    4. PE stats matmuls vs the 1/count grade matrix give per-(block,grade)
       mean and mean-of-squares: PSUM S12 [36, 2, 512].
    5. Chain on ACT/DVE/Pool: rstd = rsqrt(|var+eps|), c2n = b/w - mean*rstd.
  back end (group g-2), PE scatter matmuls interleaved into the front end's
  transpose/stats stream so the PE never waits on PSUM drain:
    6. PE scatter-matmuls expand rstd/c2n to per-element scale A / shift B.
    7. DVE 2-pass AXPY per j: tmp = x*A; out = tmp + B (in-place), DMA out.
"""

import os
import sys

if "/opt/trn_rl_repo" not in sys.path:
    sys.path.insert(0, "/opt/trn_rl_repo")

import numpy as np

BLOCK_BITS = 8
MV = 256
NG = 9
NB = 4
D = 1024
EPS = 1e-5
N_CORES = 8
TOTAL_TOKENS = 16 * 4096
TOK_PER_CORE = TOTAL_TOKENS // N_CORES  # 8192

GROUP_T = 512          # tokens per stats group
LAG = 2                # back end lags this many groups behind the front end


def _grade(m):
    return bin(m).count("1")


def _build_consts():
    import math
    counts = np.array([math.comb(8, g) for g in range(NG)], dtype=np.float32)

    # G_mean[h][i, b*9+g] = 1/count_g  for chunk h (features 128h..128h+127)
    gmean = np.zeros((8, 128, 36), dtype=np.float32)
    for h in range(8):
        b = h // 2
        for i in range(128):
            m = (h % 2) * 128 + i
            g = _grade(m)
            gmean[h, i, b * 9 + g] = 1.0 / counts[g]

    # G01[b*9+g, c] = 1 if feature c belongs to (block b, grade g)
    g01 = np.zeros((36, D), dtype=np.float32)
    for c in range(D):
        b = c // MV
        g = _grade(c % MV)
        g01[b * 9 + g, c] = 1.0

    # rstd mask: count-1 grades (0 and 8) have centered value exactly 0 in
    # the reference, so any scale works -- force rstd=0 there to avoid
    # amplifying f32r rounding by rsqrt(eps).
    mask = np.ones((36, 1), dtype=np.float32)
    for b in range(NB):
        mask[b * 9 + 0, 0] = 0.0
        mask[b * 9 + 8, 0] = 0.0
    return gmean, g01, mask


def build_nc(tok_per_core=TOK_PER_CORE):
    import concourse.bass as bass
    import concourse.tile as tile
    from concourse import bacc, mybir

    f32 = mybir.dt.float32
    f32r = mybir.dt.float32r
    bf16 = mybir.dt.bfloat16
    AF = mybir.ActivationFunctionType
    ALU = mybir.AluOpType

    gmean_np, g01_np, mask_np = _build_consts()
    n_groups = tok_per_core // GROUP_T
    assert tok_per_core % GROUP_T == 0

    nc = bacc.Bacc()
    x_d = nc.dram_tensor("x", [tok_per_core, D], f32, kind="ExternalInput")
    w_d = nc.dram_tensor("weight", [NG], f32, kind="ExternalInput")
    b_d = nc.dram_tensor("bias", [NG], f32, kind="ExternalInput")
    out_d = nc.dram_tensor("out", [tok_per_core, D], f32, kind="ExternalOutput")

    gmean_dram = nc.inline_tensor(gmean_np, name="gmean_const")
    g01_dram = nc.inline_tensor(g01_np, name="g01_const")
    ident_dram = nc.inline_tensor(np.eye(128, dtype=np.float32), name="ident_const")
    mask_dram = nc.inline_tensor(mask_np, name="mask_const")

    from contextlib import ExitStack

    with tile.TileContext(nc) as tc, ExitStack() as ctx:
        consts = ctx.enter_context(tc.tile_pool(name="consts", bufs=1))
        xg_pool = ctx.enter_context(tc.tile_pool(name="xg", bufs=5))
        xb_pool = ctx.enter_context(tc.tile_pool(name="xb", bufs=2))
        xt_pool = ctx.enter_context(tc.tile_pool(name="xt", bufs=2))
        sqt_pool = ctx.enter_context(tc.tile_pool(name="sqt", bufs=2))
        tmp_pool = ctx.enter_context(tc.tile_pool(name="tmp", bufs=2))
        small_pool = ctx.enter_context(tc.tile_pool(name="small", bufs=2))
        ps_xt = ctx.enter_context(tc.tile_pool(name="ps_xt", bufs=2, space="PSUM"))
        ps_stats = ctx.enter_context(tc.tile_pool(name="ps_st", bufs=1, space="PSUM"))
        ps_a = ctx.enter_context(tc.tile_pool(name="ps_a", bufs=1, space="PSUM"))
        ps_b = ctx.enter_context(tc.tile_pool(name="ps_b", bufs=1, space="PSUM"))

        # ---- constants into SBUF ----
        # All const DMAs go through gpsimd (SWDGE, single queue -> single
        # semaphore) so downstream compute needs at most one new wait.
        # bf16 identity + bf16 x: bf16 transposes run 1 cyc/row (vs 2 for
        # fp32) and bf16 stationaries load via a separate, overlapped
        # LDWEIGHTS (4-byte stationaries are self-loading and serialize)
        ident_f = consts.tile([128, 128], f32)
        nc.gpsimd.dma_start(out=ident_f, in_=ident_dram[:])
        ident = consts.tile([128, 128], bf16)
        nc.vector.tensor_scalar_mul(ident, ident_f, 1.0)

        gmean_f = consts.tile([128, 8, 36], f32)
        nc.gpsimd.dma_start(out=gmean_f, in_=gmean_dram[:].rearrange("h p c -> p h c"))

        g01_sb = consts.tile([36, D], f32)
        nc.gpsimd.dma_start(out=g01_sb, in_=g01_dram[:])

        # weight/bias broadcast to 36 partitions: partition p = b*9+g reads w[g]
        w36 = consts.tile([36, 1], f32)
        b36 = consts.tile([36, 1], f32)
        wap = w_d[:]
        bap = b_d[:]
        nc.gpsimd.dma_start(
            out=w36, in_=bass.AP(tensor=wap.tensor, offset=wap.offset,
                                 ap=[[0, NB]] + list(wap.ap)))
        nc.gpsimd.dma_start(
            out=b36, in_=bass.AP(tensor=bap.tensor, offset=bap.offset,
                                 ap=[[0, NB]] + list(bap.ap)))

        mask36 = consts.tile([36, 1], f32)
        nc.gpsimd.dma_start(out=mask36, in_=mask_dram[:])
        # eps + 1e38*(1-mask): count-1 grades get a huge bias so the fused
        # abs-rsqrt returns ~1e-19 (i.e. rstd ~= 0) for them
        eps36 = consts.tile([36, 1], f32)
        nc.vector.tensor_scalar(
            out=eps36, in0=mask36, scalar1=-1e38, scalar2=1e38 + EPS,
            op0=ALU.mult, op1=ALU.add)
        gmean_sb = consts.tile([128, 8, 36], bf16)
        nc.vector.tensor_scalar_mul(gmean_sb, gmean_f, 1.0)
        rw36 = consts.tile([36, 1], f32)
        nc.vector.reciprocal(rw36, w36)
        # GA[bg, c] = w[g(c)] * indicator; ga_mask additionally zeroes
        # count-1 grades (their centered value is exactly 0 in the reference)
        ga_sb = consts.tile([36, D], bf16)
        nc.vector.tensor_scalar_mul(ga_sb, g01_sb, w36)
        w36m = consts.tile([36, 1], f32)
        nc.vector.tensor_scalar_mul(w36m, w36, mask36)
        ga_mask = consts.tile([36, D], bf16)
        nc.vector.tensor_scalar_mul(ga_mask, g01_sb, w36m)
        bw36 = consts.tile([36, 1], f32)   # b/w
        nc.vector.tensor_scalar_mul(bw36, b36, rw36)
        # bw broadcast along tokens so the chain's c2n is a cheap
        # tensor_tensor on Pool (tensor_scalar with a vector scalar is a
        # microcoded slow path there)
        bwb = consts.tile([36, GROUP_T], f32)
        nc.vector.tensor_scalar(
            out=bwb, in0=g01_sb[:, 0:GROUP_T], scalar1=0.0, scalar2=bw36,
            op0=ALU.mult, op1=ALU.add)

        # ---- pipelined main loop ----
        # iteration i: prefetch dma(i+1), front end(i), back end(i-LAG)
        state = {}  # per-group tiles carried across iterations

        def dma_in(g):
            xg = xg_pool.tile([128, NB, D], f32, name="xg_t")
            nc.sync.dma_start(
                out=xg,
                in_=x_d[g * GROUP_T:(g + 1) * GROUP_T, :]
                .rearrange("(j p) d -> p j d", p=128),
            )
            state[g] = {"xg": xg}

        # token blocks j0/j2 get a bf16 copy (ACT has slack): their
        # transposes then run 1 cyc/row with an overlapped LDWEIGHTS.
        # j1/j3 transpose straight from fp32 (2 cyc/row, self-loading LDW).
        BF_J = {0: 0, 2: 1}   # j -> slot in xb

        def front_convert(g, j):
            st = state[g]
            if "xb" not in st:
                st["xb"] = xb_pool.tile([128, 2, D], bf16, name="xb_t")
            nc.scalar.copy(out=st["xb"][:, BF_J[j], :], in_=st["xg"][:, j, :])

        def front_transposes(g, half, jj):
            """Transpose feature chunks half*4..half*4+3 for token blocks jj."""
            st = state[g]
            if "xT" not in st:
                st["xT"] = xt_pool.tile([128, 8, GROUP_T], bf16, name="xT_t")
                st["sqT"] = sqt_pool.tile([128, 8, GROUP_T], bf16, name="sqT_t")
            for j in jj:
                bf = j in BF_J
                dt = bf16 if bf else f32
                xt_ps = ps_xt.tile([128, 512], dt, name="xt_ps_t")
                for cc in range(4):
                    chunk = half * 4 + cc
                    if bf:
                        src_ap = st["xb"][:, BF_J[j],
                                          chunk * 128:(chunk + 1) * 128]
                        idn = ident
                    else:
                        src_ap = st["xg"][:, j, chunk * 128:(chunk + 1) * 128]
                        idn = ident_f
                    nc.tensor.transpose(
                        xt_ps[:, cc * 128:(cc + 1) * 128], src_ap, idn)
                nc.scalar.copy(
                    out=st["xT"][:, half * 4:(half + 1) * 4,
                                 j * 128:(j + 1) * 128],
                    in_=xt_ps[:].rearrange("p (c t) -> p c t", c=4))

        def front_square(g, j, engine):
            st = state[g]
            sl = (slice(None), slice(0, 8), slice(j * 128, (j + 1) * 128))
            if engine == "act":
                nc.scalar.square(out=st["sqT"][sl], in_=st["xT"][sl])
            else:
                nc.gpsimd.tensor_tensor(
                    out=st["sqT"][sl], in0=st["xT"][sl], in1=st["xT"][sl],
                    op=ALU.mult)

        def front_stats_x(g, hs):
            st = state[g]
            if "S12" not in st:
                st["S12"] = ps_stats.tile([36, 2, GROUP_T], f32, name="S12_t")
            for h in hs:
                nc.tensor.matmul(
                    st["S12"][:, 0, :], gmean_sb[:, h, :], st["xT"][:, h, :],
                    start=(h == 0), stop=(h == 7),
                )

        def front_stats_sq(g):
            st = state[g]
            for h in range(8):
                nc.tensor.matmul(
                    st["S12"][:, 1, :], gmean_sb[:, h, :], st["sqT"][:, h, :],
                    start=(h == 0), stop=(h == 7),
                )

        def front_chain(g):
            st = state[g]
            S12 = st["S12"]
            mean_sb = small_pool.tile([36, GROUP_T], f32)
            nc.scalar.copy(out=mean_sb, in_=S12[:, 0, :])
            msq = small_pool.tile([36, GROUP_T], f32)
            nc.scalar.square(out=msq, in_=S12[:, 0, :])
            var_t = small_pool.tile([36, GROUP_T], f32)
            nc.vector.tensor_tensor(out=var_t, in0=S12[:, 1, :], in1=msq,
                                    op=ALU.subtract)
            # rstd = 1/sqrt(|var + eps|): abs also absorbs tiny negative var
            # from rounding (count-1 grades are masked out anyway).
            # bf16 so the scatter matmuls' stationary loads go through a
            # separate LDWEIGHTS that overlaps the previous matmul (4-byte
            # stationaries are self-loading and serialize on the PE).
            rstd_t = small_pool.tile([36, GROUP_T], bf16)
            nc.scalar.activation(rstd_t, var_t, AF.Abs_reciprocal_sqrt,
                                 bias=eps36, scale=1.0)
            c_t = small_pool.tile([36, GROUP_T], f32)
            nc.gpsimd.tensor_tensor(out=c_t, in0=mean_sb, in1=rstd_t,
                                    op=ALU.mult)
            # c2n = b/w - mean*rstd
            c2n_t = small_pool.tile([36, GROUP_T], bf16)
            nc.gpsimd.tensor_tensor(out=c2n_t, in0=bwb, in1=c_t,
                                    op=ALU.subtract)
            st["rstd"] = rstd_t
            st["c2n"] = c2n_t

        def back_scatter_a(g, j):
            st = state[g]
            pa = ps_a.tile([128, 2, 512], f32, name="pa_t")
            for half in range(2):
                nc.tensor.matmul(
                    pa[:, half, :], st["rstd"][:, j * 128:(j + 1) * 128],
                    ga_mask[:, half * 512:(half + 1) * 512])
            st["pa"] = pa

        def back_scatter_b(g, j):
            st = state[g]
            pb = ps_b.tile([128, 2, 512], f32, name="pb_t")
            for half in range(2):
                nc.tensor.matmul(
                    pb[:, half, :], st["c2n"][:, j * 128:(j + 1) * 128],
                    ga_sb[:, half * 512:(half + 1) * 512])
            st["pb"] = pb

        def back_pass1(g, j):
            st = state[g]
            tmp = tmp_pool.tile([128, D], f32, name="tmp_t")
            nc.vector.tensor_tensor(
                out=tmp, in0=st["xg"][:, j, :],
                in1=st["pa"][:].rearrange("p a b -> p (a b)"), op=ALU.mult)
            st["tmp"] = tmp

        def back_pass2(g, j):
            st = state[g]
            nc.vector.tensor_tensor(
                out=st["xg"][:, j, :], in0=st["tmp"],
                in1=st["pb"][:].rearrange("p a b -> p (a b)"), op=ALU.add)

        def dma_out(g):
            st = state[g]
            nc.sync.dma_start(
                out=out_d[g * GROUP_T:(g + 1) * GROUP_T, :]
                .rearrange("(j p) d -> p j d", p=128),
                in_=st["xg"],
            )
            del state[g]

        dma_in(0)
        for j in BF_J:
            front_convert(0, j)
        for i in range(n_groups + LAG):
            F = i < n_groups        # front-end group
            b = i - LAG             # back-end group
            B = b >= 0
            if i + 1 < n_groups:
                dma_in(i + 1)
            # back-end j-blocks [A(j), B(j)] + [pass1(j), pass2(j)] are
            # emitted tight so the DVE runs its two passes back-to-back;
            # B(j)'s psum-pool wait (pass2(j-1)) is hidden by the PE filler
            # (transposes/stats of the front-end group) between blocks.
            if B:
                back_scatter_a(b, 0)
                back_scatter_b(b, 0)
                back_pass1(b, 0)
                back_pass2(b, 0)
            if F:
                front_transposes(i, 0, (0, 1))
            if B:
                back_scatter_a(b, 1)
                back_scatter_b(b, 1)
                back_pass1(b, 1)
                back_pass2(b, 1)
            if F:
                front_transposes(i, 0, (2, 3))
            if B:
                back_scatter_a(b, 2)
                back_scatter_b(b, 2)
                back_pass1(b, 2)
                back_pass2(b, 2)
            if F:
                front_transposes(i, 1, (0, 1))
                front_stats_x(i, range(0, 4))
                front_square(i, 0, "act")
                front_square(i, 1, "pool")
            if B:
                back_scatter_a(b, 3)
                back_scatter_b(b, 3)
                back_pass1(b, 3)
                back_pass2(b, 3)
            if F:
                front_transposes(i, 1, (2, 3))
                front_stats_x(i, range(4, 8))
                front_square(i, 2, "pool")
                front_square(i, 3, "pool")
                front_stats_sq(i)
                if i + 1 < n_groups:
                    for j in BF_J:
                        front_convert(i + 1, j)
            if B:
                dma_out(b)
            if F:
                front_chain(i)

    nc.finalize()
    return nc


_NC_CACHE = {}


def _get_nc(tok_per_core=TOK_PER_CORE):
    key = tok_per_core
    if key not in _NC_CACHE:
        _NC_CACHE[key] = build_nc(tok_per_core)
    return _NC_CACHE[key]


def kernel(x, weight, bias, _trace=False):
    x = np.ascontiguousarray(np.asarray(x, dtype=np.float32))
    weight = np.ascontiguousarray(np.asarray(weight, dtype=np.float32))
    bias = np.ascontiguousarray(np.asarray(bias, dtype=np.float32))
    orig_shape = x.shape
    xf = x.reshape(TOTAL_TOKENS, D)

    nc = _get_nc()
    from concourse.bass_utils import run_bass_kernel_spmd

    in_maps = [
        {
            "x": np.ascontiguousarray(xf[i * TOK_PER_CORE:(i + 1) * TOK_PER_CORE]),
            "weight": weight,
            "bias": bias,
        }
        for i in range(N_CORES)
    ]
    res = run_bass_kernel_spmd(nc, in_maps, core_ids=list(range(N_CORES)),
                               trace=_trace)
    out = np.concatenate([r["out"] for r in res.results], axis=0)
    if _trace:
        kernel.last_result = res
    return out.reshape(orig_shape)


# revision 14
# speedup vs baseline: 1.2058x; 1.0995x over previous
"""CliffordLayerNorm Trainium2 kernel.

x: [16, 4096, 1024] fp32. Each row's 1024 features = 4 blocks of 256
multivector components; components are grouped into 9 grades by popcount of
their index within the block.  Per (token, block, grade): mean/var, then
out = (x - mean) * w[g] * rsqrt(var + eps) + b[g].

Strategy (per NeuronCore, data-parallel over tokens across 8 cores):
  Software-pipelined over groups of 512 tokens (16 groups/core), with the
  back end lagged 2 groups behind the front end so the tiny stats chain is
  never on the PE critical path:

  front end (group g):
    1. DMA in [128 tok, 4 j, 1024 feat] (prefetched one group ahead).
    2. PE-transposes (f32r, 1.5 cyc/row) each 128x128 chunk into PSUM.
    3. ACT copies PSUM -> SBUF xT (bf16); squares on ACT+Pool -> sqT.
    4. PE stats matmuls vs the 1/count grade matrix give per-(block,grade)
       mean and mean-of-squares: PSUM S12 [36, 2, 512].
    5. Chain on ACT/DVE/Pool: rstd = rsqrt(|var+eps|), c2n = b/w - mean*rstd.
  back end (group g-2), PE scatter matmuls interleaved into the front end's
  transpose/stats stream so the PE never waits on PSUM drain:
    6. PE scatter-matmuls expand rstd/c2n to per-element scale A / shift B.
    7. DVE 2-pass AXPY per j: tmp = x*A; out = tmp + B (in-place), DMA out.
"""

import os
import sys

if "/opt/trn_rl_repo" not in sys.path:
    sys.path.insert(0, "/opt/trn_rl_repo")

import numpy as np

BLOCK_BITS = 8
MV = 256
NG = 9
NB = 4
D = 1024
EPS = 1e-5
N_CORES = 8
TOTAL_TOKENS = 16 * 4096
TOK_PER_CORE = TOTAL_TOKENS // N_CORES  # 8192

GROUP_T = 512          # tokens per stats group
LAG = 2                # back end lags this many groups behind the front end


def _grade(m):
    return bin(m).count("1")


def _build_consts():
    import math
    counts = np.array([math.comb(8, g) for g in range(NG)], dtype=np.float32)

    # G_mean[h][i, b*9+g] = 1/count_g  for chunk h (features 128h..128h+127)
    gmean = np.zeros((8, 128, 36), dtype=np.float32)
    for h in range(8):
        b = h // 2
        for i in range(128):
            m = (h % 2) * 128 + i
            g = _grade(m)
            gmean[h, i, b * 9 + g] = 1.0 / counts[g]

    # G01[b*9+g, c] = 1 if feature c belongs to (block b, grade g)
    g01 = np.zeros((36, D), dtype=np.float32)
    for c in range(D):
        b = c // MV
        g = _grade(c % MV)
        g01[b * 9 + g, c] = 1.0

    # rstd mask: count-1 grades (0 and 8) have centered value exactly 0 in
    # the reference, so any scale works -- force rstd=0 there to avoid
    # amplifying f32r rounding by rsqrt(eps).
    mask = np.ones((36, 1), dtype=np.float32)
    for b in range(NB):
        mask[b * 9 + 0, 0] = 0.0
        mask[b * 9 + 8, 0] = 0.0
    return gmean, g01, mask


def build_nc(tok_per_core=TOK_PER_CORE):
    import concourse.bass as bass
    import concourse.tile as tile
    from concourse import bacc, mybir

    f32 = mybir.dt.float32
    f32r = mybir.dt.float32r
    bf16 = mybir.dt.bfloat16
    AF = mybir.ActivationFunctionType
    ALU = mybir.AluOpType

    gmean_np, g01_np, mask_np = _build_consts()
    n_groups = tok_per_core // GROUP_T
    assert tok_per_core % GROUP_T == 0

    nc = bacc.Bacc()
    x_d = nc.dram_tensor("x", [tok_per_core, D], f32, kind="ExternalInput")
    w_d = nc.dram_tensor("weight", [NG], f32, kind="ExternalInput")
    b_d = nc.dram_tensor("bias", [NG], f32, kind="ExternalInput")
    out_d = nc.dram_tensor("out", [tok_per_core, D], f32, kind="ExternalOutput")

    gmean_dram = nc.inline_tensor(gmean_np, name="gmean_const")
    g01_dram = nc.inline_tensor(g01_np, name="g01_const")
    ident_dram = nc.inline_tensor(np.eye(128, dtype=np.float32), name="ident_const")
    mask_dram = nc.inline_tensor(mask_np, name="mask_const")

    from contextlib import ExitStack

    with tile.TileContext(nc) as tc, ExitStack() as ctx:
        consts = ctx.enter_context(tc.tile_pool(name="consts", bufs=1))
        xg_pool = ctx.enter_context(tc.tile_pool(name="xg", bufs=5))
        xb_pool = ctx.enter_context(tc.tile_pool(name="xb", bufs=2))
        xt_pool = ctx.enter_context(tc.tile_pool(name="xt", bufs=2))
        sqt_pool = ctx.enter_context(tc.tile_pool(name="sqt", bufs=2))
        tmp_pool = ctx.enter_context(tc.tile_pool(name="tmp", bufs=2))
        small_pool = ctx.enter_context(tc.tile_pool(name="small", bufs=2))
        ps_xt = ctx.enter_context(tc.tile_pool(name="ps_xt", bufs=2, space="PSUM"))
        ps_stats = ctx.enter_context(tc.tile_pool(name="ps_st", bufs=1, space="PSUM"))
        ps_a = ctx.enter_context(tc.tile_pool(name="ps_a", bufs=1, space="PSUM"))
        ps_b = ctx.enter_context(tc.tile_pool(name="ps_b", bufs=1, space="PSUM"))

        # ---- constants into SBUF ----
        # All const DMAs go through gpsimd (SWDGE, single queue -> single
        # semaphore) so downstream compute needs at most one new wait.
        # bf16 identity + bf16 x: bf16 transposes run 1 cyc/row (vs 2 for
        # fp32) and bf16 stationaries load via a separate, overlapped
        # LDWEIGHTS (4-byte stationaries are self-loading and serialize)
        ident_f = consts.tile([128, 128], f32)
        nc.gpsimd.dma_start(out=ident_f, in_=ident_dram[:])
        ident = consts.tile([128, 128], bf16)
        nc.vector.tensor_scalar_mul(ident, ident_f, 1.0)

        gmean_f = consts.tile([128, 8, 36], f32)
        nc.gpsimd.dma_start(out=gmean_f, in_=gmean_dram[:].rearrange("h p c -> p h c"))

        g01_sb = consts.tile([36, D], f32)
        nc.gpsimd.dma_start(out=g01_sb, in_=g01_dram[:])

        # weight/bias broadcast to 36 partitions: partition p = b*9+g reads w[g]
        w36 = consts.tile([36, 1], f32)
        b36 = consts.tile([36, 1], f32)
        wap = w_d[:]
        bap = b_d[:]
        nc.gpsimd.dma_start(
            out=w36, in_=bass.AP(tensor=wap.tensor, offset=wap.offset,
                                 ap=[[0, NB]] + list(wap.ap)))
        nc.gpsimd.dma_start(
            out=b36, in_=bass.AP(tensor=bap.tensor, offset=bap.offset,
                                 ap=[[0, NB]] + list(bap.ap)))

        mask36 = consts.tile([36, 1], f32)
        nc.gpsimd.dma_start(out=mask36, in_=mask_dram[:])
        # eps + 1e38*(1-mask): count-1 grades get a huge bias so the fused
        # abs-rsqrt returns ~1e-19 (i.e. rstd ~= 0) for them
        eps36 = consts.tile([36, 1], f32)
        nc.vector.tensor_scalar(
            out=eps36, in0=mask36, scalar1=-1e38, scalar2=1e38 + EPS,
            op0=ALU.mult, op1=ALU.add)
        gmean_sb = consts.tile([128, 8, 36], bf16)
        nc.vector.tensor_scalar_mul(gmean_sb, gmean_f, 1.0)
        rw36 = consts.tile([36, 1], f32)
        nc.vector.reciprocal(rw36, w36)
        # GA[bg, c] = w[g(c)] * indicator; ga_mask additionally zeroes
        # count-1 grades (their centered value is exactly 0 in the reference)
        ga_sb = consts.tile([36, D], bf16)
        nc.vector.tensor_scalar_mul(ga_sb, g01_sb, w36)
        w36m = consts.tile([36, 1], f32)
        nc.vector.tensor_scalar_mul(w36m, w36, mask36)
        ga_mask = consts.tile([36, D], bf16)
        nc.vector.tensor_scalar_mul(ga_mask, g01_sb, w36m)
        bw36 = consts.tile([36, 1], f32)   # b/w
        nc.vector.tensor_scalar_mul(bw36, b36, rw36)
        # bw broadcast along tokens so the chain's c2n is a cheap
        # tensor_tensor on Pool (tensor_scalar with a vector scalar is a
        # microcoded slow path there)
        bwb = consts.tile([36, GROUP_T], f32)
        nc.vector.tensor_scalar(
            out=bwb, in0=g01_sb[:, 0:GROUP_T], scalar1=0.0, scalar2=bw36,
            op0=ALU.mult, op1=ALU.add)

        # ---- pipelined main loop ----
        # iteration i: prefetch dma(i+1), front end(i), back end(i-LAG)
        state = {}  # per-group tiles carried across iterations

        def dma_in(g):
            xg = xg_pool.tile([128, NB, D], f32, name="xg_t")
            nc.sync.dma_start(
                out=xg,
                in_=x_d[g * GROUP_T:(g + 1) * GROUP_T, :]
                .rearrange("(j p) d -> p j d", p=128),
            )
            state[g] = {"xg": xg}

        # token blocks j0/j2 get a bf16 copy (ACT has slack): their
        # transposes then run 1 cyc/row with an overlapped LDWEIGHTS.
        # j1/j3 transpose straight from fp32 (2 cyc/row, self-loading LDW).
        BF_J = {0: 0, 2: 1}   # j -> slot in xb

        def front_convert(g, j):
            st = state[g]
            if "xb" not in st:
                st["xb"] = xb_pool.tile([128, 2, D], bf16, name="xb_t")
            nc.scalar.copy(out=st["xb"][:, BF_J[j], :], in_=st["xg"][:, j, :])

        def front_transposes(g, half, jj):
            """Transpose feature chunks half*4..half*4+3 for token blocks jj."""
            st = state[g]
            if "xT" not in st:
                st["xT"] = xt_pool.tile([128, 8, GROUP_T], bf16, name="xT_t")
                st["sqT"] = sqt_pool.tile([128, 8, GROUP_T], bf16, name="sqT_t")
            for j in jj:
                bf = j in BF_J
                dt = bf16 if bf else f32
                xt_ps = ps_xt.tile([128, 512], dt, name="xt_ps_t")
                for cc in range(4):
                    chunk = half * 4 + cc
                    if bf:
                        src_ap = st["xb"][:, BF_J[j],
                                          chunk * 128:(chunk + 1) * 128]
                        idn = ident
                    else:
                        src_ap = st["xg"][:, j, chunk * 128:(chunk + 1) * 128]
                        idn = ident_f
                    nc.tensor.transpose(
                        xt_ps[:, cc * 128:(cc + 1) * 128], src_ap, idn)
                nc.scalar.copy(
                    out=st["xT"][:, half * 4:(half + 1) * 4,
                                 j * 128:(j + 1) * 128],
                    in_=xt_ps[:].rearrange("p (c t) -> p c t", c=4))

        def front_square(g, j, engine):
            st = state[g]
            sl = (slice(None), slice(0, 8), slice(j * 128, (j + 1) * 128))
            if engine == "act":
                nc.scalar.square(out=st["sqT"][sl], in_=st["xT"][sl])
            else:
                nc.gpsimd.tensor_tensor(
                    out=st["sqT"][sl], in0=st["xT"][sl], in1=st["xT"][sl],
                    op=ALU.mult)

        def front_stats_x(g, hs):
            st = state[g]
            if "S12" not in st:
                st["S12"] = ps_stats.tile([36, 2, GROUP_T], f32, name="S12_t")
            for h in hs:
                nc.tensor.matmul(
                    st["S12"][:, 0, :], gmean_sb[:, h, :], st["xT"][:, h, :],
                    start=(h == 0), stop=(h == 7),
                )

        def front_stats_sq(g, jp):
            # token-half split so the first half's matmuls only wait on the
            # j0/j1 squares (j2/j3 land later on the Pool queue)
            st = state[g]
            sl = slice(jp * 256, (jp + 1) * 256)
            for h in range(8):
                nc.tensor.matmul(
                    st["S12"][:, 1, sl], gmean_sb[:, h, :],
                    st["sqT"][:, h, sl],
                    start=(h == 0), stop=(h == 7),
                )

        def front_chain(g):
            st = state[g]
            S12 = st["S12"]
            msq = small_pool.tile([36, GROUP_T], f32)
            nc.scalar.square(out=msq, in_=S12[:, 0, :])
            var_t = small_pool.tile([36, GROUP_T], f32)
            nc.vector.tensor_tensor(out=var_t, in0=S12[:, 1, :], in1=msq,
                                    op=ALU.subtract)
            # rstd = 1/sqrt(|var + eps|): abs also absorbs tiny negative var
            # from rounding (count-1 grades are masked out anyway).
            # bf16 so the scatter matmuls' stationary loads go through a
            # separate LDWEIGHTS that overlaps the previous matmul (4-byte
            # stationaries are self-loading and serialize on the PE).
            rstd_t = small_pool.tile([36, GROUP_T], bf16)
            nc.scalar.activation(rstd_t, var_t, AF.Abs_reciprocal_sqrt,
                                 bias=eps36, scale=1.0)
            c_t = small_pool.tile([36, GROUP_T], f32)
            nc.vector.tensor_tensor(out=c_t, in0=S12[:, 0, :], in1=rstd_t,
                                    op=ALU.mult)
            # c2n = b/w - mean*rstd
            c2n_t = small_pool.tile([36, GROUP_T], bf16)
            nc.gpsimd.tensor_tensor(out=c2n_t, in0=bwb, in1=c_t,
                                    op=ALU.subtract)
            st["rstd"] = rstd_t
            st["c2n"] = c2n_t

        def back_scatter_a(g, j):
            st = state[g]
            pa = ps_a.tile([128, 2, 512], f32, name="pa_t")
            for half in range(2):
                nc.tensor.matmul(
                    pa[:, half, :], st["rstd"][:, j * 128:(j + 1) * 128],
                    ga_mask[:, half * 512:(half + 1) * 512])
            st["pa"] = pa

        def back_scatter_b(g, j):
            st = state[g]
            pb = ps_b.tile([128, 2, 512], f32, name="pb_t")
            for half in range(2):
                nc.tensor.matmul(
                    pb[:, half, :], st["c2n"][:, j * 128:(j + 1) * 128],
                    ga_sb[:, half * 512:(half + 1) * 512])
            st["pb"] = pb

        def back_pass1(g, j):
            st = state[g]
            tmp = tmp_pool.tile([128, D], f32, name="tmp_t")
            nc.vector.tensor_tensor(
                out=tmp, in0=st["xg"][:, j, :],
                in1=st["pa"][:].rearrange("p a b -> p (a b)"), op=ALU.mult)
            st["tmp"] = tmp

        def back_pass2(g, j):
            st = state[g]
            nc.vector.tensor_tensor(
                out=st["xg"][:, j, :], in0=st["tmp"],
                in1=st["pb"][:].rearrange("p a b -> p (a b)"), op=ALU.add)

        def dma_out(g):
            st = state[g]
            nc.sync.dma_start(
                out=out_d[g * GROUP_T:(g + 1) * GROUP_T, :]
                .rearrange("(j p) d -> p j d", p=128),
                in_=st["xg"],
            )
            del state[g]

        dma_in(0)
        for j in BF_J:
            front_convert(0, j)
        for i in range(n_groups + LAG):
            F = i < n_groups        # front-end group
            b = i - LAG             # back-end group
            B = b >= 0
            if i + 1 < n_groups:
                dma_in(i + 1)
            # back-end j-blocks [A(j), B(j)] + [pass1(j), pass2(j)] are
            # emitted tight so the DVE runs its two passes back-to-back;
            # B(j)'s psum-pool wait (pass2(j-1)) is hidden by the PE filler
            # (transposes/stats of the front-end group) between blocks.
            if B:
                back_scatter_a(b, 0)
                back_scatter_b(b, 0)
                back_pass1(b, 0)
                back_pass2(b, 0)
            if F:
                front_transposes(i, 0, (0, 1))
            if B:
                back_scatter_a(b, 1)
                back_scatter_b(b, 1)
                back_pass1(b, 1)
                back_pass2(b, 1)
            if F:
                front_transposes(i, 0, (2, 3))
            if B:
                back_scatter_a(b, 2)
                back_scatter_b(b, 2)
                back_pass1(b, 2)
                back_pass2(b, 2)
            if F:
                front_transposes(i, 1, (0, 1))
                front_stats_x(i, range(0, 4))
                front_square(i, 0, "act")
                front_square(i, 1, "pool")
            if B:
                back_scatter_a(b, 3)
                back_scatter_b(b, 3)
                back_pass1(b, 3)
                back_pass2(b, 3)
            if F:
                front_transposes(i, 1, (2, 3))
                front_stats_x(i, range(4, 8))
                front_square(i, 2, "act")
                front_square(i, 3, "pool")
                front_stats_sq(i, 0)
                front_stats_sq(i, 1)
                if i + 1 < n_groups:
                    for j in BF_J:
                        front_convert(i + 1, j)
            if B:
                dma_out(b)
            if F:
                front_chain(i)

    nc.finalize()
    return nc


_NC_CACHE = {}


def _get_nc(tok_per_core=TOK_PER_CORE):
    key = tok_per_core
    if key not in _NC_CACHE:
        _NC_CACHE[key] = build_nc(tok_per_core)
    return _NC_CACHE[key]


def kernel(x, weight, bias, _trace=False):
    x = np.ascontiguousarray(np.asarray(x, dtype=np.float32))
    weight = np.ascontiguousarray(np.asarray(weight, dtype=np.float32))
    bias = np.ascontiguousarray(np.asarray(bias, dtype=np.float32))
    orig_shape = x.shape
    xf = x.reshape(TOTAL_TOKENS, D)

    nc = _get_nc()
    from concourse.bass_utils import run_bass_kernel_spmd

    in_maps = [
        {
            "x": np.ascontiguousarray(xf[i * TOK_PER_CORE:(i + 1) * TOK_PER_CORE]),
            "weight": weight,
            "bias": bias,
        }
        for i in range(N_CORES)
    ]
    res = run_bass_kernel_spmd(nc, in_maps, core_ids=list(range(N_CORES)),
                               trace=_trace)
    out = np.concatenate([r["out"] for r in res.results], axis=0)
    if _trace:
        kernel.last_result = res
    return out.reshape(orig_shape)
